# revision 25
# baseline (speedup 1.0000x reference)
"""Causal self-attention (B=4, T=2048, C=1024, H=16) on 8 TRN2 NeuronCores.

Sharding: 8 cores = 4 batches x 2 head-groups (Megatron tensor-parallel over
heads + data-parallel over batch). Each core computes, for its batch b and its
8 heads, qkv projection -> causal attention -> its partial output projection.
Host sums the two partial outputs per batch and adds b_proj.

Schedule: one fused instruction stream. The attention jt-units (QK matmul ->
exp -> PV matmul) are inherently Scalar-engine(exp)-paced: ~1.03us of ACT work
per 512-col j-tile vs ~850ns of PE work. So independent "filler" matmul groups
(next band's qkv projection, previous bands' output projection) are sprinkled
INTO the jt stream at an even pace, sized per band to cover its exp deficit:
  band 0: s1(1)            band 1: s1(2)
  band 2: s1(3)            band 3: proj(0)+proj(1)+proj(2)
Weight loads ride the second (Activation) HWDGE queue in parallel with x loads
on the SP queue; x is prefetched two bands ahead.

dtypes: q/k/v/att/y bf16 (moving-operand bf16 keeps every attention matmul at
1 cycle/row; fp32r drops to 4 cyc/row below 256 free columns), x/wqk/wv f32r,
wp bf16 (matmul dtype pairing with bf16 y), all accumulation f32 in PSUM.
exp needs no max-subtraction: logits are O(1) by construction, 1/sqrt(hd)
folded into w_q on the host. Causality at 128-column granularity; the in-tile
triangle is applied by accumulating a -1e5 triangular bf16 mask into PSUM via
an identity matmul. Softmax sums come free from an appended ones-column on V;
normalization is reciprocal + partition_broadcast on the unnormalized PV
output (the e=1 head needs a 64->127 partition-shift DMA, DVE is lane-locked).
"""

import sys

if "/opt/trn_rl_repo" not in sys.path:
    sys.path.insert(0, "/opt/trn_rl_repo")

from contextlib import ExitStack

import numpy as np
import ml_dtypes

import concourse.bass as bass
import concourse.tile as tile
from concourse import bacc, mybir
from concourse.bass_utils import run_bass_kernel_spmd

F32 = mybir.dt.float32
F32R = mybir.dt.float32r
BF16 = mybir.dt.bfloat16
AF = mybir.ActivationFunctionType

B, T, C = 4, 2048, 1024
H, HD = 16, 64
NHL = 8          # heads per core (local)
NPAIR = 4        # head pairs per core
P = 128
TQ = 512         # query tile (free dim)
TJ = 128         # key tile (partitions)
NIT = T // TQ    # 4 query tile bands
NTS = T // P     # 16 token sub-tiles
NCT = C // P     # 8 contraction tiles over C
NEG = -100000.0  # additive causal mask value


def build_kernel(trace_label=None):
    nc = bacc.Bacc("TRN2", target_bir_lowering=False)

    xt = nc.declare_dram_parameter("xt", [NIT, P, NCT * TQ], BF16, isOutput=False)
    wqk = nc.declare_dram_parameter("wqk", [P, 8, 1024], BF16, isOutput=False)
    bqk = nc.declare_dram_parameter("bqk", [P, 8], F32, isOutput=False)
    wv = nc.declare_dram_parameter("wv", [P, NCT, 512], BF16, isOutput=False)
    bv = nc.declare_dram_parameter("bv", [1, 512], F32, isOutput=False)
    wp = nc.declare_dram_parameter("wp", [P, NPAIR, 1024], BF16, isOutput=False)
    tri = nc.declare_dram_parameter("tri", [P, P], BF16, isOutput=False)
    idn = nc.declare_dram_parameter("idn", [P, P], BF16, isOutput=False)
    out = nc.declare_dram_parameter("out", [T, C], F32, isOutput=True)

    with tile.TileContext(nc) as tc, ExitStack() as ctx:
        persist = ctx.enter_context(tc.tile_pool(name="persist", bufs=1))

        q_sb = persist.tile([P, NPAIR, T], BF16)   # queries; later y^T (reuse)
        k_sb = persist.tile([P, NPAIR, T], BF16)
        v_sb = persist.tile([P, NTS, NHL, HD + 1], BF16)
        bqk_sb = persist.tile([P, 8], F32)
        bv_sb = persist.tile([P, 512], F32)
        tri_sb = persist.tile([P, P], BF16)
        idn_sb = persist.tile([P, P], BF16)
        wqk_sb = persist.tile([P, 8, 1024], BF16)
        wv_sb = persist.tile([P, NCT, 512], BF16)
        wp_sb = persist.tile([P, NPAIR, 1024], BF16)

        s1x = ctx.enter_context(tc.tile_pool(name="s1x", bufs=3))
        attp = ctx.enter_context(tc.tile_pool(name="attp", bufs=6))
        nrm = ctx.enter_context(tc.tile_pool(name="nrm", bufs=2))
        s3o = ctx.enter_context(tc.tile_pool(name="s3o", bufs=4))
        mmps = ctx.enter_context(tc.tile_pool(name="mmps", bufs=2, space="PSUM"))
        qkps = ctx.enter_context(tc.tile_pool(name="qkps", bufs=2, space="PSUM"))
        pvps = ctx.enter_context(tc.tile_pool(name="pvps", bufs=2, space="PSUM"))

        xb_tiles = {}

        def load_x(t, eng=None):
            # one DMA per band; x is host-tiled so the band is contiguous
            # (a strided AP here fragments into 64B descriptors and hits the
            # 7ns/descriptor floor: 7.6us instead of 2.9us per band)
            xi = s1x.tile([P, NCT, TQ], BF16, tag="xc", name=f"x_{t}")
            (eng or nc.sync).dma_start(xi.rearrange("p c t -> p (c t)"), xt[t])
            xb_tiles[t] = xi

        # x(0) leads the Activation HWDGE queue (the sync queue starts with
        # two const-tensor loads); wqk streams per m-chunk behind it so the
        # t=0 accumulation groups pace with DMA arrival
        load_x(0, eng=nc.scalar)
        nc.scalar.dma_start(wqk_sb[:, 0, :], wqk[:, 0, :])
        nc.scalar.dma_start(bqk_sb, bqk[:])
        nc.scalar.dma_start(tri_sb, tri[:])
        nc.scalar.dma_start(idn_sb, idn[:])
        nc.scalar.dma_start(bv_sb[0:1, :], bv[:])
        nc.gpsimd.partition_broadcast(bv_sb[:, :], bv_sb[0:1, :])
        nc.vector.memset(v_sb[:, :, :, HD : HD + 1], 1.0)
        for i in range(1, NCT):
            nc.scalar.dma_start(wqk_sb[:, i, :], wqk[:, i, :])
        nc.scalar.dma_start(wv_sb, wv[:])
        nc.scalar.dma_start(wp_sb, wp[:])
        load_x(1)

        def emit_s1_qk(t, m):
            # q (m 0-3) / k (m 4-7) feature block: out [f-part, t-free]
            ps = mmps.tile([P, TQ], F32, tag="mm", name=f"s1qk_{t}_{m}")
            for c in range(NCT):
                nc.tensor.matmul(
                    ps,
                    wqk_sb[:, m, c * P : (c + 1) * P],
                    xb_tiles[t][:, c, :],
                    start=(c == 0),
                    stop=(c == NCT - 1),
                )
            dst = q_sb if m < 4 else k_sb
            nc.vector.tensor_scalar_add(
                dst[:, m % 4, t * TQ : (t + 1) * TQ], ps, bqk_sb[:, m : m + 1]
            )

        def emit_s1_v(t, s):
            # v block: out [t-part, f-free(head-major)] + bias, ones col kept
            ps = mmps.tile([P, 512], F32, tag="mm", name=f"s1v_{t}_{s}")
            for c in range(NCT):
                nc.tensor.matmul(
                    ps,
                    xb_tiles[t][:, c, s * P : (s + 1) * P],
                    wv_sb[:, c, :],
                    start=(c == 0),
                    stop=(c == NCT - 1),
                )
            nc.vector.tensor_tensor(
                v_sb[:, t * 4 + s, :, 0:HD],
                ps.rearrange("p (h d) -> p h d", h=NHL),
                bv_sb.rearrange("p (h d) -> p h d", h=NHL),
                mybir.AluOpType.add,
            )

        proj_n = [0]

        def emit_proj(tt, ot, tail=False):
            # in the tail (attention done) alternate psum between the mm and
            # the freed pv pools: 4-deep rotation hides the copy-sem latency
            if tail and proj_n[0] % 2:
                ps = pvps.tile([P, 512], F32, tag="pv", name=f"s3_{tt}_{ot}")
            else:
                ps = mmps.tile([P, 512], F32, tag="mm", name=f"s3_{tt}_{ot}")
            for a in range(NPAIR):
                nc.tensor.matmul(
                    ps,
                    q_sb[:, a, tt * P : (tt + 1) * P],
                    wp_sb[:, a, ot * 512 : (ot + 1) * 512],
                    start=(a == 0),
                    stop=(a == NPAIR - 1),
                )
            ot_sb = s3o.tile([P, 512], F32, tag="osb", name=f"o_{tt}_{ot}")
            nc.vector.tensor_copy(ot_sb, ps)
            eng = nc.scalar if proj_n[0] % 2 else nc.sync
            proj_n[0] += 1
            eng.dma_start(out[tt * P : (tt + 1) * P, ot * 512 : (ot + 1) * 512], ot_sb)

        def normalize(a, it, pv):
            i0 = it * TQ
            # e=1 first: its chain is longer (partition-shift DMA at the end).
            # partition_broadcast only reads partition 0, so the reciprocal
            # (lane-locked to the sums row 64) hops 64->0 via DMA; the two
            # chains ride different HWDGE queues.
            for e in (1, 0):
                rb = nrm.tile([P, TQ], F32, tag="rb", name=f"rb_{a}_{it}_{e}")
                nc.vector.reciprocal(rb[HD : HD + 1, :], pv[e][HD : HD + 1, :])
                eng = nc.sync if e else nc.scalar
                eng.dma_start(rb[0:1, :], rb[HD : HD + 1, :])
                nc.gpsimd.partition_broadcast(rb[0:HD, :], rb[0:1, :])
                if e == 0:
                    nc.vector.tensor_mul(
                        q_sb[0:HD, a, i0 : i0 + TQ], pv[e][0:HD, :], rb[0:HD, :]
                    )
                else:
                    yt = nrm.tile([P, TQ], BF16, tag="yt", name=f"yt_{a}_{it}")
                    nc.vector.tensor_mul(yt[0:HD, :], pv[e][0:HD, :], rb[0:HD, :])
                    nc.scalar.dma_start(q_sb[64:128, a, i0 : i0 + TQ], yt[0:HD, :])

        def emit_attn_band(it, extra):
            i0 = it * TQ
            njt = (i0 + TQ) // TJ
            total_jt = njt * NPAIR
            nfill = len(extra)
            filled = 0
            jcount = 0
            for a in range(NPAIR):
                pv = [
                    pvps.tile([P, TQ], F32, tag="pv", name=f"pv{e}_{a}_{it}")
                    for e in (0, 1)
                ]
                prev = None
                for jt in range(njt):
                    j0 = jt * TJ
                    d = j0 - i0
                    istart = max(d, 0)
                    nn = TQ - istart
                    # one 2-bank psum tile holds both heads' S^T blocks;
                    # a single fused exp call halves ACT instruction count
                    qk = qkps.tile([P, 2, TQ], F32, tag="qk", name=f"qk_{a}_{it}_{jt}")
                    for e in (0, 1):
                        nc.tensor.matmul(
                            qk[:, e, istart:TQ],
                            k_sb[64 * e : 64 * e + 64, a, j0 : j0 + TJ],
                            q_sb[64 * e : 64 * e + 64, a, i0 + istart : i0 + TQ],
                            start=True,
                            stop=(d < 0),
                            tile_position=(64 * e, 0),
                        )
                        if d >= 0:
                            nc.tensor.matmul(
                                qk[:, e, istart : istart + TJ],
                                idn_sb,
                                tri_sb,
                                start=False,
                                stop=True,
                                tile_position=(0, 0),
                            )
                    att = attp.tile(
                        [P, 2, TQ], BF16, tag="att", name=f"att_{a}_{it}_{jt}"
                    )
                    nc.scalar.activation(att[:, :, 0:nn], qk[:, :, istart:TQ], AF.Exp)
                    if prev is not None:
                        pjt, patt, pnn, pistart = prev
                        for e in (0, 1):
                            nc.tensor.matmul(
                                pv[e][0 : HD + 1, pistart:TQ],
                                v_sb[:, pjt, 2 * a + e, :],
                                patt[:, e, 0:pnn],
                                start=(pjt == 0),
                                stop=(pjt == njt - 1),
                            )
                    prev = (jt, att, nn, istart)
                    jcount += 1
                    # even fractional pacing of fillers over the jt stream
                    while filled < nfill and filled < (jcount * nfill) // total_jt:
                        extra[filled]()
                        filled += 1
                pjt, patt, pnn, pistart = prev
                for e in (0, 1):
                    nc.tensor.matmul(
                        pv[e][0 : HD + 1, pistart:TQ],
                        v_sb[:, pjt, 2 * a + e, :],
                        patt[:, e, 0:pnn],
                        start=(pjt == 0),
                        stop=(pjt == njt - 1),
                    )
                normalize(a, it, pv)
            while filled < nfill:
                extra[filled]()
                filled += 1

        def s1_units(t):
            return [lambda m=m: emit_s1_qk(t, m) for m in range(8)] + [
                lambda s=s: emit_s1_v(t, s) for s in range(4)
            ]

        def proj_units(tb):
            return [
                lambda tt=tt, ot=ot: emit_proj(tt, ot)
                for tt in range(tb * 4, tb * 4 + 4)
                for ot in range(2)
            ]

        # band 0 stage 1 runs standalone (nothing to overlap with yet)
        for u in s1_units(0):
            u()

        # filler assignment sized to each band's exp deficit (see module doc)
        fillers = {
            0: s1_units(1),
            1: s1_units(2),
            2: s1_units(3),
            3: proj_units(0) + proj_units(1) + proj_units(2),
        }
        for it in range(NIT):
            if it + 2 < NIT:
                load_x(it + 2)
            emit_attn_band(it, fillers[it])

        for tt in range(12, 16):
            for ot in range(2):
                emit_proj(tt, ot, tail=True)

    nc.compile()
    return nc


_NC_CACHE = None


def _get_nc():
    global _NC_CACHE
    if _NC_CACHE is None:
        _NC_CACHE = build_kernel()
    return _NC_CACHE


def _shard_inputs(x, w_qkv, b_qkv, w_proj):
    """Build the 8 per-core input maps. Core id = 2*batch + head_group."""
    tri_np = np.where(
        np.arange(P)[None, :] >= np.arange(P)[:, None], 0.0, NEG
    ).astype(ml_dtypes.bfloat16)
    idn_np = np.eye(P, dtype=ml_dtypes.bfloat16)

    in_maps = []
    for b in range(B):
        # band-contiguous tiles: xt[t, p, c*512+j] = x[b][t*512+j, c*128+p]
        xt = np.ascontiguousarray(
            x[b].reshape(NIT, TQ, NCT, P).transpose(0, 3, 2, 1)
        ).reshape(NIT, P, NCT * TQ).astype(ml_dtypes.bfloat16)
        for g in range(2):
            s = slice(g * 512, (g + 1) * 512)
            wqk_full = np.concatenate(
                [w_qkv[0:1024][s] / 8.0, w_qkv[1024:2048][s]], axis=0
            )  # [1024 f, 1024 c]
            # m-major chunks: wqk_arr[p, m, c*128+j] = wqk_full[m*128+j, c*128+p]
            wqk_arr = np.ascontiguousarray(
                wqk_full.T.reshape(NCT, P, 8, P).transpose(1, 2, 0, 3).reshape(P, 8, 1024)
            )
            bqk_full = np.concatenate([b_qkv[0:1024][s] / 8.0, b_qkv[1024:2048][s]])
            bqk_arr = np.ascontiguousarray(bqk_full.reshape(8, P).T)
            wv_rows = w_qkv[2048:3072][s]  # [512 f, 1024 c]
            wv_arr = np.ascontiguousarray(
                wv_rows.T.reshape(NCT, P, 512).transpose(1, 0, 2)
            )
            bv_arr = np.ascontiguousarray(b_qkv[2048:3072][s][None, :])
            wp_rhs = w_proj[:, s].T  # [512 hd, 1024 o]
            wp_arr = np.ascontiguousarray(
                wp_rhs.reshape(NPAIR, P, 1024).transpose(1, 0, 2)
            )
            in_maps.append(
                {
                    "xt": xt,
                    "wqk": wqk_arr.astype(ml_dtypes.bfloat16),
                    "bqk": bqk_arr.astype(np.float32),
                    "wv": wv_arr.astype(ml_dtypes.bfloat16),
                    "bv": bv_arr.astype(np.float32),
                    "wp": wp_arr.astype(ml_dtypes.bfloat16),
                    "tri": tri_np,
                    "idn": idn_np,
                }
            )
    return in_maps


def kernel(x, w_qkv, b_qkv, w_proj, b_proj, _trace=False, _trace_kwargs=None):
    x = np.asarray(x, dtype=np.float32)
    w_qkv = np.asarray(w_qkv, dtype=np.float32)
    b_qkv = np.asarray(b_qkv, dtype=np.float32)
    w_proj = np.asarray(w_proj, dtype=np.float32)
    b_proj = np.asarray(b_proj, dtype=np.float32)

    nc = _get_nc()
    in_maps = _shard_inputs(x, w_qkv, b_qkv, w_proj)
    res = run_bass_kernel_spmd(
        nc, in_maps, core_ids=list(range(8)), trace=_trace,
        **(_trace_kwargs or {}),
    )
    out = np.empty((B, T, C), np.float32)
    for b in range(B):
        out[b] = res.results[2 * b]["out"] + res.results[2 * b + 1]["out"] + b_proj
    if _trace:
        return out, res
    return out


# revision 32
# speedup vs baseline: 1.0091x; 1.0091x over previous
"""Causal self-attention (B=4, T=2048, C=1024, H=16) on 8 TRN2 NeuronCores.

Sharding: 8 cores = 4 batches x 2 head-groups (Megatron tensor-parallel over
heads + data-parallel over batch). Each core computes, for its batch b and its
8 heads, qkv projection -> causal attention -> its partial output projection.
Host sums the two partial outputs per batch and adds b_proj.

Schedule: one fused instruction stream. The attention jt-units (QK matmul ->
exp -> PV matmul) are inherently Scalar-engine(exp)-paced: ~1.03us of ACT work
per 512-col j-tile vs ~850ns of PE work. So independent "filler" matmul groups
(next band's qkv projection, previous bands' output projection) are sprinkled
INTO the jt stream at an even pace, sized per band to cover its exp deficit:
  band 0: s1(1)            band 1: s1(2)
  band 2: s1(3)            band 3: proj(0)+proj(1)+proj(2)
Weight loads ride the second (Activation) HWDGE queue in parallel with x loads
on the SP queue; x is prefetched two bands ahead.

dtypes: q/k/v/att/y bf16 (moving-operand bf16 keeps every attention matmul at
1 cycle/row; fp32r drops to 4 cyc/row below 256 free columns), x/wqk/wv f32r,
wp bf16 (matmul dtype pairing with bf16 y), all accumulation f32 in PSUM.
exp needs no max-subtraction: logits are O(1) by construction, 1/sqrt(hd)
folded into w_q on the host. Causality at 128-column granularity; the in-tile
triangle is applied by accumulating a -1e5 triangular bf16 mask into PSUM via
an identity matmul. Softmax sums come free from an appended ones-column on V;
normalization is reciprocal + partition_broadcast on the unnormalized PV
output (the e=1 head needs a 64->127 partition-shift DMA, DVE is lane-locked).
"""

import sys

if "/opt/trn_rl_repo" not in sys.path:
    sys.path.insert(0, "/opt/trn_rl_repo")

from contextlib import ExitStack

import numpy as np
import ml_dtypes

import concourse.bass as bass
import concourse.tile as tile
from concourse import bacc, mybir
from concourse.bass_utils import run_bass_kernel_spmd

F32 = mybir.dt.float32
F32R = mybir.dt.float32r
BF16 = mybir.dt.bfloat16
AF = mybir.ActivationFunctionType

B, T, C = 4, 2048, 1024
H, HD = 16, 64
NHL = 8          # heads per core (local)
NPAIR = 4        # head pairs per core
P = 128
TQ = 512         # query tile (free dim)
TJ = 128         # key tile (partitions)
NIT = T // TQ    # 4 query tile bands
NTS = T // P     # 16 token sub-tiles
NCT = C // P     # 8 contraction tiles over C
NEG = -100000.0  # additive causal mask value


def build_kernel(trace_label=None):
    nc = bacc.Bacc("TRN2", target_bir_lowering=False)

    xt = nc.declare_dram_parameter("xt", [NIT, P, NCT * TQ], BF16, isOutput=False)
    wqk = nc.declare_dram_parameter("wqk", [P, 8, 1024], BF16, isOutput=False)
    bqk = nc.declare_dram_parameter("bqk", [P, 8], F32, isOutput=False)
    wv = nc.declare_dram_parameter("wv", [P, NCT, 512], BF16, isOutput=False)
    bv = nc.declare_dram_parameter("bv", [1, 512], F32, isOutput=False)
    wp = nc.declare_dram_parameter("wp", [P, NPAIR, 1024], BF16, isOutput=False)
    tri = nc.declare_dram_parameter("tri", [P, P], BF16, isOutput=False)
    idn = nc.declare_dram_parameter("idn", [P, P], BF16, isOutput=False)
    out = nc.declare_dram_parameter("out", [T, C], F32, isOutput=True)

    with tile.TileContext(nc) as tc, ExitStack() as ctx:
        persist = ctx.enter_context(tc.tile_pool(name="persist", bufs=1))

        q_sb = persist.tile([P, NPAIR, T], BF16)   # queries; later y^T (reuse)
        k_sb = persist.tile([P, NPAIR, T], BF16)
        v_sb = persist.tile([P, NTS, NHL, HD + 1], BF16)
        bqk_sb = persist.tile([P, 8], F32)
        bv_sb = persist.tile([P, 512], F32)
        tri_sb = persist.tile([P, P], BF16)
        idn_sb = persist.tile([P, P], BF16)
        wqk_sb = persist.tile([P, 8, 1024], BF16)
        wv_sb = persist.tile([P, NCT, 512], BF16)
        wp_sb = persist.tile([P, NPAIR, 1024], BF16)

        # bufs=2: x(t+2)'s DMA is then WAR-gated on s1(t) finishing, which
        # keeps prefetches from hogging the DMA engine at startup
        s1x = ctx.enter_context(tc.tile_pool(name="s1x", bufs=2))
        attp = ctx.enter_context(tc.tile_pool(name="attp", bufs=6))
        nrm = ctx.enter_context(tc.tile_pool(name="nrm", bufs=2))
        s3o = ctx.enter_context(tc.tile_pool(name="s3o", bufs=4))
        mmps = ctx.enter_context(tc.tile_pool(name="mmps", bufs=2, space="PSUM"))
        qkps = ctx.enter_context(tc.tile_pool(name="qkps", bufs=2, space="PSUM"))
        pvps = ctx.enter_context(tc.tile_pool(name="pvps", bufs=2, space="PSUM"))

        xb_tiles = {}

        def load_x(t, eng=None):
            # one DMA per band; x is host-tiled so the band is contiguous
            # (a strided AP here fragments into 64B descriptors and hits the
            # 7ns/descriptor floor: 7.6us instead of 2.9us per band)
            xi = s1x.tile([P, NCT, TQ], BF16, tag="xc", name=f"x_{t}")
            (eng or nc.sync).dma_start(xi.rearrange("p c t -> p (c t)"), xt[t])
            xb_tiles[t] = xi

        # x(0) leads the Activation HWDGE queue (the sync queue starts with
        # two const-tensor loads); wqk streams per m-chunk behind it so the
        # t=0 accumulation groups pace with DMA arrival
        load_x(0, eng=nc.scalar)
        nc.scalar.dma_start(wqk_sb[:, 0, :], wqk[:, 0, :])
        nc.scalar.dma_start(wqk_sb[:, 1, :], wqk[:, 1, :])
        nc.scalar.dma_start(bqk_sb, bqk[:])
        nc.scalar.dma_start(tri_sb, tri[:])
        nc.scalar.dma_start(idn_sb, idn[:])
        nc.scalar.dma_start(bv_sb[0:1, :], bv[:])
        nc.gpsimd.partition_broadcast(bv_sb[:, :], bv_sb[0:1, :])
        nc.vector.memset(v_sb[:, :, :, HD : HD + 1], 1.0)
        for i in range(2, NCT):
            nc.scalar.dma_start(wqk_sb[:, i, :], wqk[:, i, :])
        load_x(1, eng=nc.scalar)
        nc.scalar.dma_start(wv_sb, wv[:])
        nc.scalar.dma_start(wp_sb, wp[:])

        def emit_s1_qk(t, m):
            # q (m 0-3) / k (m 4-7) feature block: out [f-part, t-free]
            ps = mmps.tile([P, TQ], F32, tag="mm", name=f"s1qk_{t}_{m}")
            for c in range(NCT):
                nc.tensor.matmul(
                    ps,
                    wqk_sb[:, m, c * P : (c + 1) * P],
                    xb_tiles[t][:, c, :],
                    start=(c == 0),
                    stop=(c == NCT - 1),
                )
            dst = q_sb if m < 4 else k_sb
            nc.vector.tensor_scalar_add(
                dst[:, m % 4, t * TQ : (t + 1) * TQ], ps, bqk_sb[:, m : m + 1]
            )

        def emit_s1_v(t, s):
            # v block: out [t-part, f-free(head-major)] + bias, ones col kept
            ps = mmps.tile([P, 512], F32, tag="mm", name=f"s1v_{t}_{s}")
            for c in range(NCT):
                nc.tensor.matmul(
                    ps,
                    xb_tiles[t][:, c, s * P : (s + 1) * P],
                    wv_sb[:, c, :],
                    start=(c == 0),
                    stop=(c == NCT - 1),
                )
            nc.vector.tensor_tensor(
                v_sb[:, t * 4 + s, :, 0:HD],
                ps.rearrange("p (h d) -> p h d", h=NHL),
                bv_sb.rearrange("p (h d) -> p h d", h=NHL),
                mybir.AluOpType.add,
            )

        proj_n = [0]

        def _proj_mm(ps, tt, ot, a_lo, a_hi):
            for a in range(a_lo, a_hi):
                nc.tensor.matmul(
                    ps,
                    q_sb[:, a, tt * P : (tt + 1) * P],
                    wp_sb[:, a, ot * 512 : (ot + 1) * 512],
                    start=(a == 0),
                    stop=(a == NPAIR - 1),
                )

        def _proj_out(ps, tt, ot):
            ot_sb = s3o.tile([P, 512], F32, tag="osb", name=f"o_{tt}_{ot}")
            nc.vector.tensor_copy(ot_sb, ps)
            eng = nc.scalar if proj_n[0] % 2 else nc.sync
            proj_n[0] += 1
            eng.dma_start(out[tt * P : (tt + 1) * P, ot * 512 : (ot + 1) * 512], ot_sb)

        def emit_proj(tt, ot, tail=False):
            # in the tail (attention done) alternate psum between the mm and
            # the freed pv pools: 4-deep rotation hides the copy-sem latency
            if tail and proj_n[0] % 2:
                ps = pvps.tile([P, 512], F32, tag="pv", name=f"s3_{tt}_{ot}")
            else:
                ps = mmps.tile([P, 512], F32, tag="mm", name=f"s3_{tt}_{ot}")
            _proj_mm(ps, tt, ot, 0, NPAIR)
            _proj_out(ps, tt, ot)

        def proj_open(tt, ot):
            # start a proj group on the a=0..2 partials mid-band so only the
            # a=3 matmul waits on the final normalize chain
            ps = mmps.tile([P, 512], F32, tag="mm", name=f"s3_{tt}_{ot}")
            _proj_mm(ps, tt, ot, 0, NPAIR - 1)
            return ps, tt, ot

        def proj_close(opened):
            ps, tt, ot = opened
            _proj_mm(ps, tt, ot, NPAIR - 1, NPAIR)
            _proj_out(ps, tt, ot)

        def normalize(a, it, pv):
            i0 = it * TQ
            # e=1 first: its chain is longer (partition-shift DMA at the end).
            # partition_broadcast only reads partition 0, so the reciprocal
            # (lane-locked to the sums row 64) hops 64->0 via DMA; the two
            # chains ride different HWDGE queues.
            for e in (1, 0):
                rb = nrm.tile([P, TQ], F32, tag="rb", name=f"rb_{a}_{it}_{e}")
                nc.vector.reciprocal(rb[HD : HD + 1, :], pv[e][HD : HD + 1, :])
                eng = nc.sync if e else nc.scalar
                eng.dma_start(rb[0:1, :], rb[HD : HD + 1, :])
                nc.gpsimd.partition_broadcast(rb[0:HD, :], rb[0:1, :])
                if e == 0:
                    nc.vector.tensor_mul(
                        q_sb[0:HD, a, i0 : i0 + TQ], pv[e][0:HD, :], rb[0:HD, :]
                    )
                else:
                    yt = nrm.tile([P, TQ], BF16, tag="yt", name=f"yt_{a}_{it}")
                    nc.vector.tensor_mul(yt[0:HD, :], pv[e][0:HD, :], rb[0:HD, :])
                    nc.scalar.dma_start(q_sb[64:128, a, i0 : i0 + TQ], yt[0:HD, :])

        def emit_attn_band(it, extra, last_extra=()):
            i0 = it * TQ
            njt = (i0 + TQ) // TJ
            # pace the regular fillers over a=0..2 (the last-a slots go to
            # last_extra, e.g. band 3's proj opens)
            total_jt = njt * (NPAIR - 1) if last_extra else njt * NPAIR
            nfill = len(extra)
            filled = 0
            jcount = 0
            opened = []
            for a in range(NPAIR):
                pv = [
                    pvps.tile([P, TQ], F32, tag="pv", name=f"pv{e}_{a}_{it}")
                    for e in (0, 1)
                ]
                prev = None
                for jt in range(njt):
                    j0 = jt * TJ
                    d = j0 - i0
                    istart = max(d, 0)
                    nn = TQ - istart
                    # one 2-bank psum tile holds both heads' S^T blocks;
                    # a single fused exp call halves ACT instruction count
                    qk = qkps.tile([P, 2, TQ], F32, tag="qk", name=f"qk_{a}_{it}_{jt}")
                    for e in (0, 1):
                        nc.tensor.matmul(
                            qk[:, e, istart:TQ],
                            k_sb[64 * e : 64 * e + 64, a, j0 : j0 + TJ],
                            q_sb[64 * e : 64 * e + 64, a, i0 + istart : i0 + TQ],
                            start=True,
                            stop=(d < 0),
                            tile_position=(64 * e, 0),
                        )
                        if d >= 0:
                            nc.tensor.matmul(
                                qk[:, e, istart : istart + TJ],
                                idn_sb,
                                tri_sb,
                                start=False,
                                stop=True,
                                tile_position=(0, 0),
                            )
                    att = attp.tile(
                        [P, 2, TQ], BF16, tag="att", name=f"att_{a}_{it}_{jt}"
                    )
                    nc.scalar.activation(att[:, :, 0:nn], qk[:, :, istart:TQ], AF.Exp)
                    if prev is not None:
                        pjt, patt, pnn, pistart = prev
                        for e in (0, 1):
                            nc.tensor.matmul(
                                pv[e][0 : HD + 1, pistart:TQ],
                                v_sb[:, pjt, 2 * a + e, :],
                                patt[:, e, 0:pnn],
                                start=(pjt == 0),
                                stop=(pjt == njt - 1),
                            )
                    prev = (jt, att, nn, istart)
                    jcount += 1
                    # even fractional pacing of fillers over the jt stream
                    while filled < nfill and filled < (jcount * nfill) // total_jt:
                        extra[filled]()
                        filled += 1
                    if last_extra and a == NPAIR - 1 and jt == njt // 2:
                        opened = [u() for u in last_extra]
                pjt, patt, pnn, pistart = prev
                for e in (0, 1):
                    nc.tensor.matmul(
                        pv[e][0 : HD + 1, pistart:TQ],
                        v_sb[:, pjt, 2 * a + e, :],
                        patt[:, e, 0:pnn],
                        start=(pjt == 0),
                        stop=(pjt == njt - 1),
                    )
                normalize(a, it, pv)
            while filled < nfill:
                extra[filled]()
                filled += 1
            return opened

        def s1_units(t):
            return [lambda m=m: emit_s1_qk(t, m) for m in range(8)] + [
                lambda s=s: emit_s1_v(t, s) for s in range(4)
            ]

        def proj_units(tb):
            return [
                lambda tt=tt, ot=ot: emit_proj(tt, ot)
                for tt in range(tb * 4, tb * 4 + 4)
                for ot in range(2)
            ]

        # band 0 stage 1 runs standalone (nothing to overlap with yet)
        for u in s1_units(0):
            u()

        # filler assignment sized to each band's exp deficit (see module doc)
        fillers = {
            0: s1_units(1),
            1: s1_units(2),
            2: s1_units(3),
            3: proj_units(0) + proj_units(1) + proj_units(2),
        }
        for it in range(NIT):
            if it + 2 < NIT:
                load_x(it + 2)
            last = (
                [lambda: proj_open(12, 0), lambda: proj_open(12, 1)]
                if it == NIT - 1
                else ()
            )
            opened = emit_attn_band(it, fillers[it], last)

        for o in opened:
            proj_close(o)
        for tt, ot in [(13, 0), (13, 1), (14, 0), (14, 1), (15, 0), (15, 1)]:
            emit_proj(tt, ot, tail=True)

    nc.compile()
    return nc


_NC_CACHE = None


def _get_nc():
    global _NC_CACHE
    if _NC_CACHE is None:
        _NC_CACHE = build_kernel()
    return _NC_CACHE


def _shard_inputs(x, w_qkv, b_qkv, w_proj):
    """Build the 8 per-core input maps. Core id = 2*batch + head_group."""
    tri_np = np.where(
        np.arange(P)[None, :] >= np.arange(P)[:, None], 0.0, NEG
    ).astype(ml_dtypes.bfloat16)
    idn_np = np.eye(P, dtype=ml_dtypes.bfloat16)

    in_maps = []
    for b in range(B):
        # band-contiguous tiles: xt[t, p, c*512+j] = x[b][t*512+j, c*128+p]
        xt = np.ascontiguousarray(
            x[b].reshape(NIT, TQ, NCT, P).transpose(0, 3, 2, 1)
        ).reshape(NIT, P, NCT * TQ).astype(ml_dtypes.bfloat16)
        for g in range(2):
            s = slice(g * 512, (g + 1) * 512)
            wqk_full = np.concatenate(
                [w_qkv[0:1024][s] / 8.0, w_qkv[1024:2048][s]], axis=0
            )  # [1024 f, 1024 c]
            # m-major chunks: wqk_arr[p, m, c*128+j] = wqk_full[m*128+j, c*128+p]
            wqk_arr = np.ascontiguousarray(
                wqk_full.T.reshape(NCT, P, 8, P).transpose(1, 2, 0, 3).reshape(P, 8, 1024)
            )
            bqk_full = np.concatenate([b_qkv[0:1024][s] / 8.0, b_qkv[1024:2048][s]])
            bqk_arr = np.ascontiguousarray(bqk_full.reshape(8, P).T)
            wv_rows = w_qkv[2048:3072][s]  # [512 f, 1024 c]
            wv_arr = np.ascontiguousarray(
                wv_rows.T.reshape(NCT, P, 512).transpose(1, 0, 2)
            )
            bv_arr = np.ascontiguousarray(b_qkv[2048:3072][s][None, :])
            wp_rhs = w_proj[:, s].T  # [512 hd, 1024 o]
            wp_arr = np.ascontiguousarray(
                wp_rhs.reshape(NPAIR, P, 1024).transpose(1, 0, 2)
            )
            in_maps.append(
                {
                    "xt": xt,
                    "wqk": wqk_arr.astype(ml_dtypes.bfloat16),
                    "bqk": bqk_arr.astype(np.float32),
                    "wv": wv_arr.astype(ml_dtypes.bfloat16),
                    "bv": bv_arr.astype(np.float32),
                    "wp": wp_arr.astype(ml_dtypes.bfloat16),
                    "tri": tri_np,
                    "idn": idn_np,
                }
            )
    return in_maps


def kernel(x, w_qkv, b_qkv, w_proj, b_proj, _trace=False, _trace_kwargs=None):
    x = np.asarray(x, dtype=np.float32)
    w_qkv = np.asarray(w_qkv, dtype=np.float32)
    b_qkv = np.asarray(b_qkv, dtype=np.float32)
    w_proj = np.asarray(w_proj, dtype=np.float32)
    b_proj = np.asarray(b_proj, dtype=np.float32)

    nc = _get_nc()
    in_maps = _shard_inputs(x, w_qkv, b_qkv, w_proj)
    res = run_bass_kernel_spmd(
        nc, in_maps, core_ids=list(range(8)), trace=_trace,
        **(_trace_kwargs or {}),
    )
    out = np.empty((B, T, C), np.float32)
    for b in range(B):
        out[b] = res.results[2 * b]["out"] + res.results[2 * b + 1]["out"] + b_proj
    if _trace:
        return out, res
    return out


# revision 41
# speedup vs baseline: 1.0173x; 1.0082x over previous
"""Causal self-attention (B=4, T=2048, C=1024, H=16) on 8 TRN2 NeuronCores.

Sharding: 8 cores = 4 batches x 2 head-groups (Megatron tensor-parallel over
heads + data-parallel over batch). Each core computes, for its batch b and its
8 heads, qkv projection -> causal attention -> its partial output projection.
Host sums the two partial outputs per batch and adds b_proj.

Schedule: one fused instruction stream. The attention jt-units (QK matmul ->
exp -> PV matmul) are inherently Scalar-engine(exp)-paced: ~1.03us of ACT work
per 512-col j-tile vs ~850ns of PE work. So independent "filler" matmul groups
(next band's qkv projection, previous bands' output projection) are sprinkled
INTO the jt stream at an even pace, sized per band to cover its exp deficit:
  band 0: s1(1)            band 1: s1(2)
  band 2: s1(3)            band 3: proj(0)+proj(1)+proj(2)
Weight loads ride the second (Activation) HWDGE queue in parallel with x loads
on the SP queue; x is prefetched two bands ahead.

dtypes: q/k/v/att/y bf16 (moving-operand bf16 keeps every attention matmul at
1 cycle/row; fp32r drops to 4 cyc/row below 256 free columns), x/wqk/wv f32r,
wp bf16 (matmul dtype pairing with bf16 y), all accumulation f32 in PSUM.
exp needs no max-subtraction: logits are O(1) by construction, 1/sqrt(hd)
folded into w_q on the host. Causality at 128-column granularity; the in-tile
triangle is applied by accumulating a -1e5 triangular bf16 mask into PSUM via
an identity matmul. Softmax sums come free from an appended ones-column on V;
normalization is reciprocal + partition_broadcast on the unnormalized PV
output (the e=1 head needs a 64->127 partition-shift DMA, DVE is lane-locked).
"""

import sys

if "/opt/trn_rl_repo" not in sys.path:
    sys.path.insert(0, "/opt/trn_rl_repo")

from contextlib import ExitStack

import numpy as np
import ml_dtypes

import concourse.bass as bass
import concourse.tile as tile
from concourse import bacc, mybir
from concourse.bass_utils import run_bass_kernel_spmd

F32 = mybir.dt.float32
F32R = mybir.dt.float32r
BF16 = mybir.dt.bfloat16
AF = mybir.ActivationFunctionType

B, T, C = 4, 2048, 1024
H, HD = 16, 64
NHL = 8          # heads per core (local)
NPAIR = 4        # head pairs per core
P = 128
TQ = 512         # query tile (free dim)
TJ = 128         # key tile (partitions)
NIT = T // TQ    # 4 query tile bands
NTS = T // P     # 16 token sub-tiles
NCT = C // P     # 8 contraction tiles over C
NEG = -100000.0  # additive causal mask value


def build_kernel(trace_label=None):
    nc = bacc.Bacc("TRN2", target_bir_lowering=False)

    xt = nc.declare_dram_parameter("xt", [NIT, P, NCT * TQ], BF16, isOutput=False)
    wqk = nc.declare_dram_parameter("wqk", [P, 8, 1024], BF16, isOutput=False)
    bqk = nc.declare_dram_parameter("bqk", [P, 8], F32, isOutput=False)
    wv = nc.declare_dram_parameter("wv", [P, NCT, 512], BF16, isOutput=False)
    bv = nc.declare_dram_parameter("bv", [1, 512], F32, isOutput=False)
    wp = nc.declare_dram_parameter("wp", [P, NPAIR, 1024], BF16, isOutput=False)
    tri = nc.declare_dram_parameter("tri", [P, P], BF16, isOutput=False)
    idn = nc.declare_dram_parameter("idn", [P, P], BF16, isOutput=False)
    out = nc.declare_dram_parameter("out", [T, C], BF16, isOutput=True)

    with tile.TileContext(nc) as tc, ExitStack() as ctx:
        persist = ctx.enter_context(tc.tile_pool(name="persist", bufs=1))

        q_sb = persist.tile([P, NPAIR, T], BF16)   # queries; later y^T (reuse)
        k_sb = persist.tile([P, NPAIR, T], BF16)
        v_sb = persist.tile([P, NTS, NHL, HD + 1], BF16)
        bqk_sb = persist.tile([P, 8], F32)
        bv_sb = persist.tile([P, 512], F32)
        tri_sb = persist.tile([P, P], BF16)
        idn_sb = persist.tile([P, P], BF16)
        wqk_sb = persist.tile([P, 8, 1024], BF16)
        wv_sb = persist.tile([P, NCT, 512], BF16)
        wp_sb = persist.tile([P, NPAIR, 1024], BF16)

        # bufs=2: x(t+2)'s DMA is then WAR-gated on s1(t) finishing, which
        # keeps prefetches from hogging the DMA engine at startup
        s1x = ctx.enter_context(tc.tile_pool(name="s1x", bufs=2))
        attp = ctx.enter_context(tc.tile_pool(name="attp", bufs=6))
        nrm = ctx.enter_context(tc.tile_pool(name="nrm", bufs=2))
        s3o = ctx.enter_context(tc.tile_pool(name="s3o", bufs=4))
        mmps = ctx.enter_context(tc.tile_pool(name="mmps", bufs=2, space="PSUM"))
        qkps = ctx.enter_context(tc.tile_pool(name="qkps", bufs=2, space="PSUM"))
        pvps = ctx.enter_context(tc.tile_pool(name="pvps", bufs=1, space="PSUM"))

        xb_tiles = {}

        def load_x(t, eng=None):
            # one DMA per band; x is host-tiled so the band is contiguous
            # (a strided AP here fragments into 64B descriptors and hits the
            # 7ns/descriptor floor: 7.6us instead of 2.9us per band)
            xi = s1x.tile([P, NCT, TQ], BF16, tag="xc", name=f"x_{t}")
            (eng or nc.sync).dma_start(xi.rearrange("p c t -> p (c t)"), xt[t])
            xb_tiles[t] = xi

        # x(0) leads the Activation HWDGE queue (the sync queue starts with
        # two const-tensor loads); wqk streams per m-chunk behind it so the
        # t=0 accumulation groups pace with DMA arrival
        load_x(0, eng=nc.scalar)
        nc.scalar.dma_start(wqk_sb[:, 0, :], wqk[:, 0, :])
        nc.scalar.dma_start(wqk_sb[:, 1, :], wqk[:, 1, :])
        nc.scalar.dma_start(bqk_sb, bqk[:])
        nc.scalar.dma_start(tri_sb, tri[:])
        nc.scalar.dma_start(idn_sb, idn[:])
        nc.scalar.dma_start(bv_sb[0:1, :], bv[:])
        nc.gpsimd.partition_broadcast(bv_sb[:, :], bv_sb[0:1, :])
        nc.vector.memset(v_sb[:, :, :, HD : HD + 1], 1.0)
        for i in range(2, NCT):
            nc.scalar.dma_start(wqk_sb[:, i, :], wqk[:, i, :])
        load_x(1, eng=nc.scalar)
        nc.scalar.dma_start(wv_sb, wv[:])
        nc.scalar.dma_start(wp_sb, wp[:])

        def emit_s1_qk(t, m):
            # q (m 0-3) / k (m 4-7) feature block: out [f-part, t-free]
            ps = mmps.tile([P, TQ], F32, tag="mm", name=f"s1qk_{t}_{m}")
            for c in range(NCT):
                nc.tensor.matmul(
                    ps,
                    wqk_sb[:, m, c * P : (c + 1) * P],
                    xb_tiles[t][:, c, :],
                    start=(c == 0),
                    stop=(c == NCT - 1),
                )
            dst = q_sb if m < 4 else k_sb
            nc.vector.tensor_scalar_add(
                dst[:, m % 4, t * TQ : (t + 1) * TQ], ps, bqk_sb[:, m : m + 1]
            )

        def emit_s1_v(t, s):
            # v block: out [t-part, f-free(head-major)] + bias, ones col kept
            ps = mmps.tile([P, 512], F32, tag="mm", name=f"s1v_{t}_{s}")
            for c in range(NCT):
                nc.tensor.matmul(
                    ps,
                    xb_tiles[t][:, c, s * P : (s + 1) * P],
                    wv_sb[:, c, :],
                    start=(c == 0),
                    stop=(c == NCT - 1),
                )
            nc.vector.tensor_tensor(
                v_sb[:, t * 4 + s, :, 0:HD],
                ps.rearrange("p (h d) -> p h d", h=NHL),
                bv_sb.rearrange("p (h d) -> p h d", h=NHL),
                mybir.AluOpType.add,
            )

        proj_n = [0]

        def _proj_mm(ps, tt, ot, a_lo, a_hi):
            for a in range(a_lo, a_hi):
                nc.tensor.matmul(
                    ps,
                    q_sb[:, a, tt * P : (tt + 1) * P],
                    wp_sb[:, a, ot * 512 : (ot + 1) * 512],
                    start=(a == 0),
                    stop=(a == NPAIR - 1),
                )

        def _proj_out(ps, tt, ot):
            ot_sb = s3o.tile([P, 512], BF16, tag="osb", name=f"o_{tt}_{ot}")
            nc.vector.tensor_copy(ot_sb, ps)
            eng = nc.scalar if proj_n[0] % 2 else nc.sync
            proj_n[0] += 1
            eng.dma_start(out[tt * P : (tt + 1) * P, ot * 512 : (ot + 1) * 512], ot_sb)

        def emit_proj(tt, ot, tail=False):
            # in the tail (attention done) alternate psum between the mm and
            # the freed pv pools (pv-shaped tile, first half used): 4-deep
            # rotation hides the copy-sem latency
            if tail and proj_n[0] % 2:
                ps = pvps.tile([P, 2, TQ], F32, tag="pv", name=f"s3_{tt}_{ot}")[
                    :, 0, :
                ]
            else:
                ps = mmps.tile([P, 512], F32, tag="mm", name=f"s3_{tt}_{ot}")
            _proj_mm(ps, tt, ot, 0, NPAIR)
            _proj_out(ps, tt, ot)

        def proj_open(tt, ot):
            # start a proj group on the a=0..2 partials mid-band so only the
            # a=3 matmul waits on the final normalize chain
            ps = mmps.tile([P, 512], F32, tag="mm", name=f"s3_{tt}_{ot}")
            _proj_mm(ps, tt, ot, 0, NPAIR - 1)
            return ps, tt, ot

        def proj_close(opened):
            ps, tt, ot = opened
            _proj_mm(ps, tt, ot, NPAIR - 1, NPAIR)
            _proj_out(ps, tt, ot)

        def normalize(a, it, pv):
            i0 = it * TQ
            # both heads' sums live on psum row 64 of the fused pv tile, so
            # one reciprocal + one 64->0 partition hop (partition_broadcast
            # only reads partition 0; DVE is lane-locked) + one broadcast
            # serve both normalizations
            rb = nrm.tile([P, 2, TQ], F32, tag="rb", name=f"rb_{a}_{it}")
            nc.vector.reciprocal(rb[HD : HD + 1, :, :], pv[HD : HD + 1, :, :])
            nc.sync.dma_start(rb[0:1, :, :], rb[HD : HD + 1, :, :])
            nc.gpsimd.partition_broadcast(rb[0:HD, :, :], rb[0:1, :, :])
            # e=1 first: its chain is longer (partition-shift DMA at the end)
            yt = nrm.tile([P, TQ], BF16, tag="yt", name=f"yt_{a}_{it}")
            nc.vector.tensor_mul(yt[0:HD, :], pv[0:HD, 1, :], rb[0:HD, 1, :])
            nc.scalar.dma_start(q_sb[64:128, a, i0 : i0 + TQ], yt[0:HD, :])
            nc.vector.tensor_mul(
                q_sb[0:HD, a, i0 : i0 + TQ], pv[0:HD, 0, :], rb[0:HD, 0, :]
            )

        def emit_attn_band(it, extra, last_extra=()):
            i0 = it * TQ
            njt = (i0 + TQ) // TJ
            # pace the regular fillers over a=0..2 (the last-a slots go to
            # last_extra, e.g. band 3's proj opens)
            total_jt = njt * (NPAIR - 1) if last_extra else njt * NPAIR
            nfill = len(extra)
            filled = 0
            jcount = 0
            opened = []
            for a in range(NPAIR):
                pv = pvps.tile([P, 2, TQ], F32, tag="pv", name=f"pv_{a}_{it}")
                prev = None
                for jt in range(njt):
                    j0 = jt * TJ
                    d = j0 - i0
                    istart = max(d, 0)
                    nn = TQ - istart
                    # one 2-bank psum tile holds both heads' S^T blocks;
                    # a single fused exp call halves ACT instruction count
                    qk = qkps.tile([P, 2, TQ], F32, tag="qk", name=f"qk_{a}_{it}_{jt}")
                    for e in (0, 1):
                        nc.tensor.matmul(
                            qk[:, e, istart:TQ],
                            k_sb[64 * e : 64 * e + 64, a, j0 : j0 + TJ],
                            q_sb[64 * e : 64 * e + 64, a, i0 + istart : i0 + TQ],
                            start=True,
                            stop=(d < 0),
                            tile_position=(64 * e, 0),
                        )
                        if d >= 0:
                            nc.tensor.matmul(
                                qk[:, e, istart : istart + TJ],
                                idn_sb,
                                tri_sb,
                                start=False,
                                stop=True,
                                tile_position=(0, 0),
                            )
                    att = attp.tile(
                        [P, 2, TQ], BF16, tag="att", name=f"att_{a}_{it}_{jt}"
                    )
                    nc.scalar.activation(att[:, :, 0:nn], qk[:, :, istart:TQ], AF.Exp)
                    if prev is not None:
                        pjt, patt, pnn, pistart = prev
                        for e in (0, 1):
                            nc.tensor.matmul(
                                pv[0 : HD + 1, e, pistart:TQ],
                                v_sb[:, pjt, 2 * a + e, :],
                                patt[:, e, 0:pnn],
                                start=(pjt == 0),
                                stop=(pjt == njt - 1),
                            )
                    prev = (jt, att, nn, istart)
                    jcount += 1
                    # even fractional pacing of fillers over the jt stream
                    while filled < nfill and filled < (jcount * nfill) // total_jt:
                        extra[filled]()
                        filled += 1
                    if last_extra and a == NPAIR - 1 and jt == njt // 2:
                        opened = [u() for u in last_extra]
                pjt, patt, pnn, pistart = prev
                for e in (0, 1):
                    nc.tensor.matmul(
                        pv[0 : HD + 1, e, pistart:TQ],
                        v_sb[:, pjt, 2 * a + e, :],
                        patt[:, e, 0:pnn],
                        start=(pjt == 0),
                        stop=(pjt == njt - 1),
                    )
                normalize(a, it, pv)
            while filled < nfill:
                extra[filled]()
                filled += 1
            return opened

        def s1_units(t):
            return [lambda m=m: emit_s1_qk(t, m) for m in range(8)] + [
                lambda s=s: emit_s1_v(t, s) for s in range(4)
            ]

        def proj_units(tb):
            return [
                lambda tt=tt, ot=ot: emit_proj(tt, ot)
                for tt in range(tb * 4, tb * 4 + 4)
                for ot in range(2)
            ]

        # band 0 stage 1 runs standalone (nothing to overlap with yet)
        for u in s1_units(0):
            u()

        # filler assignment sized to each band's exp deficit (see module doc)
        fillers = {
            0: s1_units(1),
            1: s1_units(2),
            2: s1_units(3),
            3: proj_units(0) + proj_units(1) + proj_units(2),
        }
        for it in range(NIT):
            if it + 2 < NIT:
                load_x(it + 2)
            last = (
                [lambda: proj_open(12, 0), lambda: proj_open(12, 1)]
                if it == NIT - 1
                else ()
            )
            opened = emit_attn_band(it, fillers[it], last)

        for o in opened:
            proj_close(o)
        for tt, ot in [(13, 0), (13, 1), (14, 0), (14, 1), (15, 0), (15, 1)]:
            emit_proj(tt, ot, tail=True)

    nc.compile()
    return nc


_NC_CACHE = None


def _get_nc():
    global _NC_CACHE
    if _NC_CACHE is None:
        _NC_CACHE = build_kernel()
    return _NC_CACHE


def _shard_inputs(x, w_qkv, b_qkv, w_proj):
    """Build the 8 per-core input maps. Core id = 2*batch + head_group."""
    tri_np = np.where(
        np.arange(P)[None, :] >= np.arange(P)[:, None], 0.0, NEG
    ).astype(ml_dtypes.bfloat16)
    idn_np = np.eye(P, dtype=ml_dtypes.bfloat16)

    in_maps = []
    for b in range(B):
        # band-contiguous tiles: xt[t, p, c*512+j] = x[b][t*512+j, c*128+p]
        xt = np.ascontiguousarray(
            x[b].reshape(NIT, TQ, NCT, P).transpose(0, 3, 2, 1)
        ).reshape(NIT, P, NCT * TQ).astype(ml_dtypes.bfloat16)
        for g in range(2):
            s = slice(g * 512, (g + 1) * 512)
            wqk_full = np.concatenate(
                [w_qkv[0:1024][s] / 8.0, w_qkv[1024:2048][s]], axis=0
            )  # [1024 f, 1024 c]
            # m-major chunks: wqk_arr[p, m, c*128+j] = wqk_full[m*128+j, c*128+p]
            wqk_arr = np.ascontiguousarray(
                wqk_full.T.reshape(NCT, P, 8, P).transpose(1, 2, 0, 3).reshape(P, 8, 1024)
            )
            bqk_full = np.concatenate([b_qkv[0:1024][s] / 8.0, b_qkv[1024:2048][s]])
            bqk_arr = np.ascontiguousarray(bqk_full.reshape(8, P).T)
            wv_rows = w_qkv[2048:3072][s]  # [512 f, 1024 c]
            wv_arr = np.ascontiguousarray(
                wv_rows.T.reshape(NCT, P, 512).transpose(1, 0, 2)
            )
            bv_arr = np.ascontiguousarray(b_qkv[2048:3072][s][None, :])
            wp_rhs = w_proj[:, s].T  # [512 hd, 1024 o]
            wp_arr = np.ascontiguousarray(
                wp_rhs.reshape(NPAIR, P, 1024).transpose(1, 0, 2)
            )
            in_maps.append(
                {
                    "xt": xt,
                    "wqk": wqk_arr.astype(ml_dtypes.bfloat16),
                    "bqk": bqk_arr.astype(np.float32),
                    "wv": wv_arr.astype(ml_dtypes.bfloat16),
                    "bv": bv_arr.astype(np.float32),
                    "wp": wp_arr.astype(ml_dtypes.bfloat16),
                    "tri": tri_np,
                    "idn": idn_np,
                }
            )
    return in_maps


def kernel(x, w_qkv, b_qkv, w_proj, b_proj, _trace=False, _trace_kwargs=None):
    x = np.asarray(x, dtype=np.float32)
    w_qkv = np.asarray(w_qkv, dtype=np.float32)
    b_qkv = np.asarray(b_qkv, dtype=np.float32)
    w_proj = np.asarray(w_proj, dtype=np.float32)
    b_proj = np.asarray(b_proj, dtype=np.float32)

    nc = _get_nc()
    in_maps = _shard_inputs(x, w_qkv, b_qkv, w_proj)
    res = run_bass_kernel_spmd(
        nc, in_maps, core_ids=list(range(8)), trace=_trace,
        **(_trace_kwargs or {}),
    )
    out = np.empty((B, T, C), np.float32)
    for b in range(B):
        out[b] = (
            res.results[2 * b]["out"].astype(np.float32)
            + res.results[2 * b + 1]["out"].astype(np.float32)
            + b_proj
        )
    if _trace:
        return out, res
    return out
